# revision 5
# baseline (speedup 1.0000x reference)
"""Multi-head attention (b=4, n=2048, dim=512, heads=8, d_head=64) on 8 TRN2 NeuronCores.

Sharding: core = 2*b + head_group. Data parallel over batch (4), tensor
parallel over heads (2 groups of 4). Each core computes QKV projection for
its 4 heads, full attention, and a partial output projection (its heads'
rows of W_out); the host sums the two partials per batch (the unshard step
of row-parallel tensor parallelism).

Device algorithm per core (all layouts chosen so no on-device transposes of
activations are needed except the tiny 128x64 U tiles):
  - inputs: xT [512,2048] (pre-transposed on host), wqkT [512,512]
    (columns = q heads then k heads), wvT [512,256], woT [256,512] bf16
  - qk^T = wqkT.T @ xT -> [512(o), 2048(n)]  (o on partitions = head-major
    d, i.e. exactly the q^T/k^T tiles attention needs)
  - v = xT.T @ wvT -> [2048, 256] natural, stored with a 1.0 column per
    head ([128, 4*65] tiles) so P~ @ [v|1] yields the softmax denominator
    for free
  - per head, per i-block of 512: S^T[j, i] = k^T.T @ q^T (f32r matmuls),
    P~ = exp(S^T * scale) via ScalarE (PSUM -> SBUF bf16, scale fused)
  - PV: U[i, 65] = P~^T.T @ [v|1] (bf16), normalize rows by 1/U[:,64]
    (VectorE reciprocal + tensor_scalar), transpose U via PE into A^T
  - out = A^T.T @ woT (bf16) -> partial [2048, 512] f32 -> DMA out
"""

import functools
import sys

if "/opt/trn_rl_repo" not in sys.path:
    sys.path.insert(0, "/opt/trn_rl_repo")

import numpy as np
import ml_dtypes

import concourse.bacc as bacc
import concourse.mybir as mybir
import concourse.tile as tile
from concourse import masks
from concourse.bass_utils import run_bass_kernel_spmd

N_CORES = 8
B = 4
N = 2048          # sequence length
C = 512           # model dim
HPC = 4           # heads per core
D = 64            # head dim
SCALE = D ** -0.5

F32 = mybir.dt.float32
F32R = mybir.dt.float32r
BF16 = mybir.dt.bfloat16

NT = N // 128     # 16 n/j tiles of 128
KT = C // 128     # 4 contraction tiles for the projections
IB = 4            # i-blocks of 512
JP = NT // 2      # 8 j-tile pairs per i-block


def _build_body(nc, tc, ctx, xT_d, wqkT_d, wvT_d, woT_d, out_d):
    sb = ctx.enter_context(tc.tile_pool(name="sb", bufs=1))
    work = ctx.enter_context(tc.tile_pool(name="work", bufs=3))
    ppool = ctx.enter_context(tc.tile_pool(name="pt", bufs=2))
    pbig = ctx.enter_context(tc.tile_pool(name="pbig", bufs=3, space="PSUM"))
    psmall = ctx.enter_context(tc.tile_pool(name="psmall", bufs=2, space="PSUM"))

    # ---- persistent SBUF tensors ----
    xT = [sb.tile([128, N], F32R, tag=f"x{k}", name=f"x{k}") for k in range(KT)]
    wqk = [sb.tile([128, 512], F32R, tag=f"wqk{k}", name=f"wqk{k}") for k in range(KT)]
    wv = [sb.tile([128, 256], F32R, tag=f"wv{k}", name=f"wv{k}") for k in range(KT)]
    wo = [sb.tile([128, 512], BF16, tag=f"wo{t}", name=f"wo{t}") for t in range(2)]
    qkT = [sb.tile([128, N], F32R, tag=f"qk{o}", name=f"qk{o}") for o in range(4)]
    vsb = [sb.tile([128, HPC * 65], BF16, tag=f"v{t}", name=f"v{t}") for t in range(NT)]
    AT = [sb.tile([128, N], BF16, tag=f"at{t}", name=f"at{t}") for t in range(2)]
    ident = sb.tile([128, 128], BF16, tag="ident")

    # ---- input DMAs (weights first: first matmuls need them) ----
    for k in range(KT):
        nc.sync.dma_start(out=wqk[k][:], in_=wqkT_d[k * 128:(k + 1) * 128, :])
    for k in range(KT):
        nc.sync.dma_start(out=wv[k][:], in_=wvT_d[k * 128:(k + 1) * 128, :])
    for t in range(2):
        nc.sync.dma_start(out=wo[t][:], in_=woT_d[t * 128:(t + 1) * 128, :])
    for k in range(KT):
        nc.sync.dma_start(out=xT[k][:], in_=xT_d[k * 128:(k + 1) * 128, :])
    masks.make_identity(nc, ident[:])

    # ones columns of v tiles (never overwritten by the v eviction)
    for t in range(NT):
        v3 = vsb[t][:].rearrange("p (h c) -> p h c", c=65)
        nc.vector.memset(v3[:, :, 64:65], 1.0)

    # ---- qk^T projection: out[o_tile, n] = wqkT.T @ xT ----
    # o-tiles: 0 = q heads 0/1, 1 = q heads 2/3, 2 = k heads 0/1, 3 = k 2/3.
    # Emit heads 0/1 (tiles 0 and 2) first so attention can start early.
    for ot in (0, 2, 1, 3):
        for nch in range(4):
            ps = pbig.tile([128, 1024], F32, tag="ps")
            for k in range(KT):
                nc.tensor.matmul(
                    ps[:, 0:512],
                    wqk[k][:, ot * 128:(ot + 1) * 128],
                    xT[k][:, nch * 512:(nch + 1) * 512],
                    start=(k == 0),
                    stop=(k == KT - 1),
                )
            nc.vector.tensor_copy(qkT[ot][:, nch * 512:(nch + 1) * 512], ps[:, 0:512])

    # ---- v projection (natural layout): v[n_tile, 256] = xT.T @ wvT ----
    for t in range(NT):
        ps = pbig.tile([128, 1024], F32, tag="ps")
        for k in range(KT):
            nc.tensor.matmul(
                ps[:, 0:256],
                xT[k][:, t * 128:(t + 1) * 128],
                wv[k][:],
                start=(k == 0),
                stop=(k == KT - 1),
            )
        v3 = vsb[t][:].rearrange("p (h c) -> p h c", c=65)
        p3 = ps[:, 0:256].rearrange("p (h c) -> p h c", c=64)
        nc.vector.tensor_copy(v3[:, :, 0:64], p3)

    # ---- attention ----
    exp_t = mybir.ActivationFunctionType.Exp
    for ib in range(IB):
        for h in range(HPC):
            rows = slice((h % 2) * 64, (h % 2) * 64 + 64)
            q_t = qkT[h // 2]
            k_t = qkT[2 + h // 2]
            rhs_q = q_t[rows, ib * 512:(ib + 1) * 512]
            ptiles = []
            for jp in range(JP):
                ps = pbig.tile([128, 1024], F32, tag="ps")
                for half in range(2):
                    jt = jp * 2 + half
                    nc.tensor.matmul(
                        ps[:, half * 512:(half + 1) * 512],
                        k_t[rows, jt * 128:(jt + 1) * 128],
                        rhs_q,
                        start=True,
                        stop=True,
                    )
                pt = ppool.tile([128, 1024], BF16, tag=f"pt{jp}", name=f"pt{jp}")
                nc.scalar.activation(pt[:], ps[:], exp_t, scale=SCALE)
                ptiles.append(pt)
            for it in range(4):
                pu = psmall.tile([128, 128], F32, tag="pu")
                for jt in range(NT):
                    lhsT = ptiles[jt // 2][:, (jt % 2) * 512 + it * 128:(jt % 2) * 512 + it * 128 + 128]
                    nc.tensor.matmul(
                        pu[:, 0:65],
                        lhsT,
                        vsb[jt][:, h * 65:h * 65 + 65],
                        start=(jt == 0),
                        stop=(jt == NT - 1),
                    )
                rc = work.tile([128, 1], F32, tag="r")
                nc.vector.reciprocal(rc[:], pu[:, 64:65])
                ut = work.tile([128, 64], BF16, tag="u")
                nc.vector.tensor_scalar_mul(ut[:], pu[:, 0:64], rc[:])
                ptr = psmall.tile([128, 128], BF16, tag="pu", name="ptr")
                nc.tensor.transpose(ptr[0:64, :], ut[:], ident[:])
                nc.vector.tensor_copy(
                    AT[h // 2][rows, ib * 512 + it * 128:ib * 512 + (it + 1) * 128],
                    ptr[0:64, :],
                )
        # ---- output projection for this i-block (all heads done) ----
        for t in range(4):
            nt = ib * 4 + t
            pp = pbig.tile([128, 1024], F32, tag="ps")
            for t2 in range(2):
                nc.tensor.matmul(
                    pp[:, 0:512],
                    AT[t2][:, nt * 128:(nt + 1) * 128],
                    wo[t2][:],
                    start=(t2 == 0),
                    stop=(t2 == 1),
                )
            ot_s = work.tile([128, 512], F32, tag="o")
            nc.vector.tensor_copy(ot_s[:], pp[:, 0:512])
            nc.sync.dma_start(out=out_d[nt * 128:(nt + 1) * 128, :], in_=ot_s[:])


@functools.lru_cache(maxsize=1)
def _build():
    nc = bacc.Bacc("TRN2", target_bir_lowering=False, debug=False,
                   num_devices=N_CORES)
    xT_d = nc.dram_tensor("xT", [C, N], F32R, kind="ExternalInput").ap()
    wqkT_d = nc.dram_tensor("wqkT", [C, 512], F32R, kind="ExternalInput").ap()
    wvT_d = nc.dram_tensor("wvT", [C, 256], F32R, kind="ExternalInput").ap()
    woT_d = nc.dram_tensor("woT", [256, C], BF16, kind="ExternalInput").ap()
    out_d = nc.dram_tensor("out", [N, C], F32, kind="ExternalOutput").ap()
    from contextlib import ExitStack
    with tile.TileContext(nc) as tc, ExitStack() as ctx:
        _build_body(nc, tc, ctx, xT_d, wqkT_d, wvT_d, woT_d, out_d)
    nc.compile()
    return nc


def _shard_inputs(x, W_qkv, W_out):
    in_maps = []
    for core in range(N_CORES):
        b, hg = core // 2, core % 2
        xT = np.ascontiguousarray(x[b].T)
        rows_q = W_qkv[hg * 256:(hg + 1) * 256, :]
        rows_k = W_qkv[512 + hg * 256:512 + (hg + 1) * 256, :]
        wqkT = np.ascontiguousarray(np.concatenate([rows_q, rows_k], 0).T)
        wvT = np.ascontiguousarray(
            W_qkv[1024 + hg * 256:1024 + (hg + 1) * 256, :].T)
        woT = np.ascontiguousarray(
            W_out[:, hg * 256:(hg + 1) * 256].T).astype(ml_dtypes.bfloat16)
        in_maps.append({"xT": xT, "wqkT": wqkT, "wvT": wvT, "woT": woT})
    return in_maps


def _run(inputs, trace=False, tmpdir=None):
    x = np.asarray(inputs["x"], dtype=np.float32)
    W_qkv = np.asarray(inputs["W_qkv"], dtype=np.float32)
    W_out = np.asarray(inputs["W_out"], dtype=np.float32)
    nc = _build()
    in_maps = _shard_inputs(x, W_qkv, W_out)
    kwargs = {}
    if trace:
        kwargs = dict(trace=True, tmpdir=tmpdir)
    res = run_bass_kernel_spmd(nc, in_maps, core_ids=list(range(N_CORES)), **kwargs)
    out = np.zeros((B, N, C), np.float32)
    for core in range(N_CORES):
        out[core // 2] += res.results[core]["out"]
    return out, res


def kernel(**inputs):
    out, _ = _run(inputs)
    return out
